# revision 1
# baseline (speedup 1.0000x reference)
"""Trainium2 Bass kernel for nn_BasicBlock (WeightNet/CondConv-style block).

Data parallel over batch: 32 samples -> 8 cores x 4 samples.
Per core, per sample:
  gap   = mean(x, HW) @ reduce_w.T + reduce_b                  (PE + DVE)
  a_wn  = sigmoid(gap @ fc1_w.T + fc1_b)                       (PE + ACT)
  W_wn  = einsum('gi,goi->go', a, w2) per-sample 3x3 kernels   (DVE)
  out   = relu(bn1(conv(x, W1)))                               (PE + ACT)
  out   = bn2(conv(out, W2)) + x; relu                         (PE + ACT + DVE)
Conv implemented as 9 shifted bf16 matmuls accumulating in PSUM, padded
58x58 image layout in SBUF. Static weights host-pre-packed (layout only).
"""

import sys

sys.path.insert(0, "/opt/trn_rl_repo")

import numpy as np
import ml_dtypes

import concourse.bass as bass
import concourse.tile as tile
from concourse import bacc, mybir
from concourse import bass_utils

F32 = mybir.dt.float32
BF16 = mybir.dt.bfloat16
AF = mybir.ActivationFunctionType

B, C, H, W = 32, 256, 56, 56
NCORES = 8
BL = B // NCORES          # samples per core
HP, WP = H + 2, W + 2     # padded 58x58
NPIX = H * W              # 3136
NPPAD = HP * WP           # 3364
NT = 7                    # h-tiles of 8 rows
TROWS = 8
NFREE = TROWS * W         # 448 columns per matmul
EPS = 1e-5


def build_program():
    nc = bacc.Bacc("TRN2", target_bir_lowering=False, debug=False,
                   num_devices=NCORES)

    x4 = nc.dram_tensor("x4", [BL, C, H, W], F32, kind="ExternalInput").ap()
    x4b = nc.dram_tensor("x4b", [BL, C, H, W], BF16, kind="ExternalInput").ap()
    out4 = nc.dram_tensor("out4", [BL, C, H, W], F32, kind="ExternalOutput").ap()
    rwT = nc.dram_tensor("rwT", [2, 128, 16], F32, kind="ExternalInput").ap()
    rb = nc.dram_tensor("rb", [16, 1], F32, kind="ExternalInput").ap()
    fc1wT = [nc.dram_tensor(f"fc1wT{n}", [16, 4096], BF16, kind="ExternalInput").ap()
             for n in (1, 2)]
    fc1b = [nc.dram_tensor(f"fc1b{n}", [128, 32], F32, kind="ExternalInput").ap()
            for n in (1, 2)]
    w2p = [nc.dram_tensor(f"w2p{n}", [2, 128, 4 * 9 * 256], BF16,
                          kind="ExternalInput").ap() for n in (1, 2)]
    bns = [nc.dram_tensor(f"bns{n}", [2, 128, 1], F32, kind="ExternalInput").ap()
           for n in (1, 2)]
    bnb = [nc.dram_tensor(f"bnb{n}", [2, 128, 1], F32, kind="ExternalInput").ap()
           for n in (1, 2)]

    with tile.TileContext(nc) as tc:
        build_body(tc, x4, x4b, out4, rwT, rb, fc1wT, fc1b, w2p, bns, bnb)

    nc.compile()
    return nc


def build_body(tc, x4, x4b, out4, rwT, rb, fc1wT, fc1b, w2p, bns, bnb):
    nc = tc.nc
    from contextlib import ExitStack
    ctx = ExitStack()

    cpool = ctx.enter_context(tc.tile_pool(name="consts", bufs=1))
    xpad_p = ctx.enter_context(tc.tile_pool(name="xpad", bufs=4))
    o1pad_p = ctx.enter_context(tc.tile_pool(name="o1pad", bufs=4))
    aexp_p = ctx.enter_context(tc.tile_pool(name="aexp", bufs=2))
    wgen_p = ctx.enter_context(tc.tile_pool(name="wgen", bufs=6))
    wtmp_p = ctx.enter_context(tc.tile_pool(name="wtmp", bufs=2))
    small_p = ctx.enter_context(tc.tile_pool(name="small", bufs=2))
    stage_p = ctx.enter_context(tc.tile_pool(name="stage", bufs=2))
    avlin_p = ctx.enter_context(tc.tile_pool(name="avlinp", bufs=1))
    xstage_p = ctx.enter_context(tc.tile_pool(name="xstage", bufs=1))
    psum_p = ctx.enter_context(tc.tile_pool(name="psum", bufs=5, space="PSUM"))
    psmall_p = ctx.enter_context(tc.tile_pool(name="psmall", bufs=1, space="PSUM"))
    dram_p = ctx.enter_context(tc.tile_pool(name="dscratch", bufs=2, space="DRAM"))

    # sample-0 chunk-0 staging load first: it heads the critical chain
    xs_pre = xstage_p.tile([128, NPIX], BF16, tag="xstage")
    nc.sync.dma_start(xs_pre[:],
                      x4b[0, 0:128].rearrange("c h w -> c (h w)"))

    # ---- resident constants (w2sb DMAs deferred for startup overlap) ----
    w2sb = []   # [wn][chunk][i] -> [128, 2304] bf16 (k*256+co)
    for n in range(2):
        per = []
        for c in range(2):
            blocks = []
            for i in range(4):
                w2t = cpool.tile([128, 2304], BF16, tag=f"w2sb{n}{c}{i}")
                blocks.append(w2t)
            per.append(blocks)
        w2sb.append(per)

    def load_w2sb(n):
        for c in range(2):
            for i in range(4):
                nc.sync.dma_start(w2sb[n][c][i][:],
                                  w2p[n][c][:, 2304 * i:2304 * (i + 1)])
    rwT_sb = []
    for c in range(2):
        t = cpool.tile([128, 16], F32, tag=f"rwT{c}")
        nc.sync.dma_start(t[:], rwT[c])
        rwT_sb.append(t)
    rb_sb = cpool.tile([16, 1], F32, tag="rb")
    nc.sync.dma_start(rb_sb[:], rb)
    fc1wT_sb, fc1b_sb, bns_sb, bnb_sb = [], [], [], []
    for n in range(2):
        t = cpool.tile([16, 4096], BF16, tag=f"fc1wT{n}")
        if n == 0:
            nc.sync.dma_start(t[:], fc1wT[n])
        fc1wT_sb.append(t)
        t = cpool.tile([128, 32], F32, tag=f"fc1b{n}")
        if n == 0:
            nc.sync.dma_start(t[:], fc1b[n])
        fc1b_sb.append(t)
        ts, tb = [], []
        for c in range(2):
            a = cpool.tile([128, 1], F32, tag=f"bns{n}{c}")
            ts.append(a)
            a = cpool.tile([128, 1], F32, tag=f"bnb{n}{c}")
            tb.append(a)
        bns_sb.append(ts)
        bnb_sb.append(tb)

    def load_deferred_consts():
        nc.sync.dma_start(fc1wT_sb[1][:], fc1wT[1])
        nc.sync.dma_start(fc1b_sb[1][:], fc1b[1])
        for n in range(2):
            for c in range(2):
                nc.sync.dma_start(bns_sb[n][c][:], bns[n][c])
                nc.sync.dma_start(bnb_sb[n][c][:], bnb[n][c])
    gap16 = cpool.tile([16, BL], BF16, tag="gap16")
    ones_sb = cpool.tile([1, 64], BF16, tag="ones")
    nc.gpsimd.memset(ones_sb[:], 1.0)

    def border_memset(t):
        r = t[:].rearrange("p (h w) -> p h w", h=HP)
        nc.gpsimd.memset(r[:, 0, :], 0.0)
        nc.gpsimd.memset(r[:, HP - 1, :], 0.0)
        nc.gpsimd.memset(r[:, 1:HP - 1, 0:1], 0.0)
        nc.gpsimd.memset(r[:, 1:HP - 1, WP - 1:WP], 0.0)

    def gen_weights_a(wn, s):
        """sigmoid(fc1(gap)) -> partition-broadcast coefficient tiles."""
        aps = psmall_p.tile([128, 32], F32, tag="avec_ps")
        for j in range(32):
            nc.tensor.matmul(aps[:, j:j + 1],
                             fc1wT_sb[wn][:, 128 * j:128 * (j + 1)],
                             gap16[:, s:s + 1],
                             start=True, stop=True)
        avt = small_p.tile([128, 32], F32, tag="avtmp")
        nc.vector.tensor_add(avt[:], aps[:], fc1b_sb[wn][:])
        avec = small_p.tile([128, 32], BF16, tag="avec")
        nc.scalar.activation(avec[:], avt[:], AF.Sigmoid)
        avd = dram_p.tile([4096], BF16, tag="avd")
        nc.sync.dma_start(avd[:].rearrange("(j p) -> p j", p=128), avec[:])
        avlin = avlin_p.tile([1, 4096], BF16, tag="avlin")
        nc.sync.dma_start(avlin[:], avd[:].unsqueeze(0))
        avr = avlin[:].rearrange("o (co r) -> o co r", r=16)
        aexp = []
        for c in range(2):
            t = aexp_p.tile([128, 4 * 256], BF16, tag=f"aexp{c}")
            for half in range(2):
                aps2 = psmall_p.tile([128, 2 * 256], F32, tag="aexp_ps")
                for h in range(2):
                    for ii in range(2):
                        i = 2 * half + ii
                        m = 4 * (2 * c + h) + i
                        rhs = avr[:, :, m:m + 1].rearrange("o co r -> o (co r)")
                        nc.tensor.matmul(
                            aps2[64 * h:64 * (h + 1), 256 * ii:256 * (ii + 1)],
                            ones_sb[:], rhs, start=True, stop=True)
                nc.scalar.copy(t[:, 512 * half:512 * (half + 1)], aps2[:])
            aexp.append(t)
        return aexp

    def gen_weights_b(wn, aexp):
        wt = []
        for c in range(2):
            t = wgen_p.tile([128, 9 * 256], BF16, tag="wgen")

            def abid(i):
                return (aexp[c][:, 256 * i:256 * (i + 1)].unsqueeze(1)
                        .broadcast_to([128, 9, 256]))

            def k3(ap2d, lo):
                return ap2d[:, lo:lo + 2304].rearrange(
                    "p (k co) -> p k co", k=9)

            nc.vector.tensor_mul(k3(t[:], 0), k3(w2sb[wn][c][0][:], 0), abid(0))
            for i in range(1, 4):
                tmp = wtmp_p.tile([128, 9 * 256], BF16, tag="wtmp")
                nc.vector.tensor_mul(
                    k3(tmp[:], 0), k3(w2sb[wn][c][i][:], 0), abid(i))
                nc.vector.tensor_add(t[:], t[:], tmp[:])
            wt.append(t)
        return wt

    def gen_weights(wn, s):
        return gen_weights_b(wn, gen_weights_a(wn, s))

    def conv(wt, src_pads, sink):
        """9-offset shifted matmul conv; sink(cc, t, psum_tile) evacuates."""
        for cc in range(2):
            for t in range(NT):
                ps = psum_p.tile([128, NFREE], F32, tag="cps")
                first = True
                for c in range(2):
                    xr = src_pads[c][:].rearrange("p (h w) -> p h w", h=HP)
                    for kh in range(3):
                        for kw in range(3):
                            k = 3 * kh + kw
                            nc.tensor.matmul(
                                ps[:],
                                wt[c][:, 256 * k + 128 * cc:
                                      256 * k + 128 * cc + 128],
                                xr[:, TROWS * t + kh:TROWS * t + kh + TROWS,
                                   kw:kw + W],
                                start=first, stop=(c == 1 and k == 8))
                            first = False
                sink(cc, t, ps)

    def load_x_gap(s, pre=None):
        xpad = []
        gsum = []
        for c in range(2):
            if c == 0 and pre is not None:
                xs = pre
            else:
                xs = xstage_p.tile([128, NPIX], BF16, tag="xstage")
                nc.sync.dma_start(
                    xs[:],
                    x4b[s, 128 * c:128 * (c + 1)].rearrange("c h w -> c (h w)"))
            g = small_p.tile([128, 1], F32, tag="gsum")
            xp = xpad_p.tile([128, NPPAD], BF16, tag="xpad")
            border_memset(xp)
            xpr = xp[:].rearrange("p (h w) -> p h w", h=HP)
            nc.scalar.activation(xpr[:, 1:1 + H, 1:1 + W],
                                 xs[:].rearrange("p (h w) -> p h w", h=H),
                                 AF.Copy, accum_out=g[:])
            xpad.append(xp)
            gsum.append(g)
        gps = psmall_p.tile([16, 1], F32, tag="gap_ps")
        for c in range(2):
            nc.tensor.matmul(gps[:], rwT_sb[c][:], gsum[c][:],
                             start=(c == 0), stop=(c == 1))
        nc.scalar.activation(gap16[:, s:s + 1], gps[:], AF.Identity,
                             bias=rb_sb[:], scale=1.0)
        return xpad

    xpad = load_x_gap(0, pre=xs_pre)
    ax0 = gen_weights_a(0, 0)
    load_w2sb(0)
    w1 = gen_weights_b(0, ax0)
    load_deferred_consts()
    load_w2sb(1)
    xpad_next = None
    w1_next = None

    for s in range(BL):
        w2 = gen_weights(1, s)
        if s + 1 < BL:
            xpad_next = load_x_gap(s + 1)
            w1_next = gen_weights(0, s + 1)

        # ---- conv1 + bn1 + relu -> o1pad (bf16, padded) ----
        o1pad = []
        for c in range(2):
            op = o1pad_p.tile([128, NPPAD], BF16, tag="o1pad")
            border_memset(op)
            o1pad.append(op)

        def sink1(cc, t, ps):
            opr = o1pad[cc][:].rearrange("p (h w) -> p h w", h=HP)
            nc.scalar.activation(
                opr[:, TROWS * t + 1:TROWS * t + 1 + TROWS, 1:1 + W],
                ps[:].rearrange("p (h w) -> p h w", h=TROWS),
                AF.Relu, bias=bnb_sb[0][cc][:], scale=bns_sb[0][cc][:])

        conv(w1, xpad, sink1)

        # ---- conv2 + bn2 + residual + relu -> out ----
        def sink2(cc, t, ps):
            t2 = stage_p.tile([128, NFREE], F32, tag="t2")
            nc.scalar.activation(t2[:], ps[:], AF.Identity,
                                 bias=bnb_sb[1][cc][:], scale=bns_sb[1][cc][:])
            xres = stage_p.tile([128, NFREE], F32, tag="xres")
            xflat = x4[s, 128 * cc:128 * (cc + 1)].rearrange("c h w -> c (h w)")
            nc.sync.dma_start(xres[:], xflat[:, NFREE * t:NFREE * (t + 1)])
            nc.vector.tensor_add(t2[:], t2[:], xres[:])
            nc.vector.tensor_scalar_max(t2[:], t2[:], 0.0)
            oflat = out4[s, 128 * cc:128 * (cc + 1)].rearrange("c h w -> c (h w)")
            nc.sync.dma_start(oflat[:, NFREE * t:NFREE * (t + 1)], t2[:])

        conv(w2, o1pad, sink2)
        xpad = xpad_next
        w1 = w1_next

    ctx.close()


_NC_CACHE = {}


def get_program():
    if "nc" not in _NC_CACHE:
        _NC_CACHE["nc"] = build_program()
    return _NC_CACHE["nc"]


def prep_inputs(inputs):
    x = np.asarray(inputs["x"], np.float32)
    f32 = lambda a: np.ascontiguousarray(np.asarray(a, np.float32))
    bf = lambda a: np.ascontiguousarray(
        np.asarray(a, np.float32).astype(ml_dtypes.bfloat16))

    def pack_w2(fc2_w):
        w2_ = np.asarray(fc2_w, np.float32).reshape(256, 4, 64, 9, 4)
        w2h = w2_.transpose(4, 3, 1, 2, 0).reshape(4, 9, 256, 256)
        return bf(w2h.transpose(2, 0, 1, 3).reshape(2, 128, 4 * 9 * 256))

    def bn_fold(g, b, m, v):
        sc = np.asarray(g, np.float32) / np.sqrt(np.asarray(v, np.float32) + EPS)
        bia = np.asarray(b, np.float32) - np.asarray(m, np.float32) * sc
        return f32(sc.reshape(2, 128, 1)), f32(bia.reshape(2, 128, 1))

    base = {
        "rwT": f32((np.asarray(inputs["reduce_w"], np.float32).T / NPIX)
                   .reshape(2, 128, 16)),
        "rb": f32(np.asarray(inputs["reduce_b"]).reshape(16, 1)),
        "fc1wT1": bf(np.asarray(inputs["w1_fc1_w"]).T),
        "fc1wT2": bf(np.asarray(inputs["w2_fc1_w"]).T),
        "fc1b1": f32(np.asarray(inputs["w1_fc1_b"]).reshape(32, 128).T),
        "fc1b2": f32(np.asarray(inputs["w2_fc1_b"]).reshape(32, 128).T),
        "w2p1": pack_w2(inputs["w1_fc2_w"]),
        "w2p2": pack_w2(inputs["w2_fc2_w"]),
    }
    base["bns1"], base["bnb1"] = bn_fold(inputs["bn1_g"], inputs["bn1_b"],
                                         inputs["bn1_m"], inputs["bn1_v"])
    base["bns2"], base["bnb2"] = bn_fold(inputs["bn2_g"], inputs["bn2_b"],
                                         inputs["bn2_m"], inputs["bn2_v"])
    xb = x.astype(ml_dtypes.bfloat16)
    in_maps = []
    for i in range(NCORES):
        m = dict(base)
        m["x4"] = np.ascontiguousarray(x[i * BL:(i + 1) * BL])
        m["x4b"] = np.ascontiguousarray(xb[i * BL:(i + 1) * BL])
        in_maps.append(m)
    return in_maps


def kernel(**inputs):
    in_maps = prep_inputs(inputs)
    nc = get_program()
    res = bass_utils.run_bass_kernel_spmd(nc, in_maps,
                                          core_ids=list(range(NCORES)))
    out = np.concatenate([r["out4"] for r in res.results], axis=0)
    return out.astype(np.float32)



# revision 17
# speedup vs baseline: 1.2275x; 1.2275x over previous
"""Trainium2 Bass kernel for nn_BasicBlock (WeightNet/CondConv-style block).

Data parallel over batch: 32 samples -> 8 cores x 4 samples.

fp8 (e4m3) 3-pass DoubleRow conv: every conv operand (x, o1, W1, W2) is split
into fp8 hi + lo (exact residual, same scale), and each 3x3 conv accumulates
  Wh@xh + Wh@xl + Wl@xh
in fp32 PSUM via DoubleRow matmuls (contraction 256 = 2x128 chunks per
instruction).  Dropping the lo*lo term keeps bf16-level accuracy at 27/36 of
the bf16 matmul row count.  Scales: x*16, W*64 (basis pre-scaled on host),
o1*2; all folded into the BN affine host-side, so all three passes accumulate
at a single PSUM scale.

Per core, per sample:
  gap   = sum(x_hi, HW) @ rwT + rb                    (ACT accum + PE)
  a     = sigmoid(fc1p(gap))  (fc1 host-permuted so the broadcast of a to
          [128, ch, i, co] is two contiguous stride-0-partition DMAs)
  W     = sum_i a_i * w2p_i   (DVE broadcast-mul chain, bf16)
  Wh/Wl = fp8 split            (Pool copy + DVE sub)
  conv1 -> bn1+relu -> o1 hi/lo (ACT x2 + DVE sub), conv2 -> bn2 fold
  out   = relu(bn2(conv2) + x) (ACT psum evac, Pool add+max)
"""

import sys

sys.path.insert(0, "/opt/trn_rl_repo")

import numpy as np
import ml_dtypes

import concourse.bass as bass
import concourse.tile as tile
from concourse import bacc, mybir
from concourse import bass_utils

F32 = mybir.dt.float32
BF16 = mybir.dt.bfloat16
F8 = mybir.dt.float8e4
AF = mybir.ActivationFunctionType
DR = mybir.MatmulPerfMode.DoubleRow
ALU = mybir.AluOpType

E4NP = ml_dtypes.float8_e4m3
BFNP = ml_dtypes.bfloat16

B, C, H, W = 32, 256, 56, 56
NCORES = 8
BL = B // NCORES          # samples per core
HP, WP = H + 2, W + 2     # padded 58x58
NPIX = H * W              # 3136
NPPAD = HP * WP           # 3364
NT = 7                    # h-tiles of 8 rows
TROWS = 8
NFREE = TROWS * W         # 448 columns per conv output tile
EPS = 1e-5
SX, SW, SO1 = 16.0, 64.0, 2.0


def build_program():
    nc = bacc.Bacc("TRN2", target_bir_lowering=False, debug=False,
                   num_devices=NCORES)

    xh8 = nc.dram_tensor("xh8p", [BL, C, HP, WP], F8, kind="ExternalInput").ap()
    xl8 = nc.dram_tensor("xl8p", [BL, C, HP, WP], F8, kind="ExternalInput").ap()
    xb2 = nc.dram_tensor("xb2", [BL, C, H, W], BF16, kind="ExternalInput").ap()
    out4 = nc.dram_tensor("out4", [BL, C, H, W], F32, kind="ExternalOutput").ap()
    rwT = nc.dram_tensor("rwT", [2, 128, 16], F32, kind="ExternalInput").ap()
    rb = nc.dram_tensor("rb", [16, 1], F32, kind="ExternalInput").ap()
    fc1wT = [nc.dram_tensor(f"fc1wTp{n}", [16, 4096], BF16,
                            kind="ExternalInput").ap() for n in (1, 2)]
    fc1b = [nc.dram_tensor(f"fc1bp{n}", [128, 32], F32,
                           kind="ExternalInput").ap() for n in (1, 2)]
    w2p = [nc.dram_tensor(f"w2p{n}", [4, 128, 2 * 9 * 256], BF16,
                          kind="ExternalInput").ap() for n in (1, 2)]
    bns = [nc.dram_tensor(f"bns{n}", [2, 128, 1], F32,
                          kind="ExternalInput").ap() for n in (1, 2)]
    bnb1 = nc.dram_tensor("bnb1", [2, 128, 1], F32, kind="ExternalInput").ap()

    with tile.TileContext(nc) as tc:
        build_body(tc, xh8, xl8, xb2, out4, rwT, rb, fc1wT, fc1b, w2p, bns,
                   bnb1)

    nc.compile()
    return nc


def build_body(tc, xh8, xl8, xb2, out4, rwT, rb, fc1wT, fc1b, w2p, bns, bnb1):
    nc = tc.nc
    from contextlib import ExitStack
    ctx = ExitStack()

    cpool = ctx.enter_context(tc.tile_pool(name="consts", bufs=1))
    wbf_p = ctx.enter_context(tc.tile_pool(name="wbf", bufs=1))
    wtmp_p = ctx.enter_context(tc.tile_pool(name="wtmp", bufs=1))
    w8_p = ctx.enter_context(tc.tile_pool(name="w8", bufs=4))
    aexp_p = ctx.enter_context(tc.tile_pool(name="aexp", bufs=2))
    small_p = ctx.enter_context(tc.tile_pool(name="small", bufs=4))
    stage_p = ctx.enter_context(tc.tile_pool(name="stage", bufs=3))
    psum_p = ctx.enter_context(tc.tile_pool(name="psum", bufs=6, space="PSUM"))
    psmall_p = ctx.enter_context(tc.tile_pool(name="psmall", bufs=1,
                                              space="PSUM"))
    dram_p = ctx.enter_context(tc.tile_pool(name="dscratch", bufs=2,
                                            space="DRAM"))

    xh_r = xh8.rearrange("s (c p) h w -> s p c (h w)", c=2)
    xl_r0 = xl8.rearrange("s (c p) h w -> s p c (h w)", c=2)

    # ACT table warmup: trigger the LoadActFuncSet at t~0, off the x path
    warm = cpool.tile([1, 1], F32, tag="warm")
    nc.gpsimd.memset(warm[:], 0.0)
    nc.scalar.activation(warm[:], warm[:], AF.Copy)

    # sample-0 image DMAs lead the queue: they head the gap->fc1->wgen chain
    xpad0h = cpool.tile([128, 2, NPPAD], F8, tag="xpadh0")
    xpad0l = cpool.tile([128, 2, NPPAD], F8, tag="xpadl0")
    for c in range(2):
        nc.sync.dma_start(xpad0h[:, c], xh_r[0, :, c])
    for c in range(2):
        nc.sync.dma_start(xpad0l[:, c], xl_r0[0, :, c])

    # ---- resident constants ----
    rwT_sb = []
    for c in range(2):
        t = cpool.tile([128, 16], F32, tag=f"rwT{c}")
        nc.sync.dma_start(t[:], rwT[c])
        rwT_sb.append(t)
    rb_sb = cpool.tile([16, 1], F32, tag="rb")
    nc.sync.dma_start(rb_sb[:], rb)

    fc1wT_sb, fc1b_sb = [], []
    for n in range(2):
        t = cpool.tile([16, 4096], BF16, tag=f"fc1wT{n}")
        if n == 0:
            nc.sync.dma_start(t[:], fc1wT[n])
        fc1wT_sb.append(t)
        t = cpool.tile([128, 32], F32, tag=f"fc1b{n}")
        if n == 0:
            nc.sync.dma_start(t[:], fc1b[n])
        fc1b_sb.append(t)

    w2sb = []   # [n][i] -> [128, 2*9*256] bf16, layout (ch, k, co)
    for n in range(2):
        per = []
        for i in range(4):
            t = cpool.tile([128, 2 * 9 * 256], BF16, tag=f"w2sb{n}{i}")
            per.append(t)
        w2sb.append(per)

    NWQ = 4   # chunks per basis-tile DMA: keeps DMA_ENGINES holds short
    def load_w2sb(n):
        q = 4608 // NWQ
        for i in range(4):
            for j in range(NWQ):
                nc.sync.dma_start(w2sb[n][i][:, q * j:q * (j + 1)],
                                  w2p[n][i][:, q * j:q * (j + 1)])

    bns_sb = []
    for n in range(2):
        per = []
        for cc in range(2):
            t = cpool.tile([128, 1], F32, tag=f"bns{n}{cc}")
            per.append(t)
        bns_sb.append(per)
    bnb1_sb = []
    for cc in range(2):
        t = cpool.tile([128, 1], F32, tag=f"bnb1{cc}")
        bnb1_sb.append(t)

    def load_bn_consts():
        for n in range(2):
            for cc in range(2):
                nc.sync.dma_start(bns_sb[n][cc][:], bns[n][cc])
        for cc in range(2):
            nc.sync.dma_start(bnb1_sb[cc][:], bnb1[cc])

    def load_deferred_consts():
        nc.sync.dma_start(fc1wT_sb[1][:], fc1wT[1])
        nc.sync.dma_start(fc1b_sb[1][:], fc1b[1])

    gap16 = cpool.tile([16, BL], BF16, tag="gap16")
    accsink = cpool.tile([128, NPPAD], F8, tag="accsink")

    # persistent padded tiles (double-buffered by sample parity);
    # borders zeroed once here, only interiors are rewritten per sample.
    def make_pads(name, nbuf):
        tiles = []
        for j in range(nbuf):
            t = cpool.tile([128, 2, NPPAD], F8, tag=f"{name}{j}")
            r = t[:].rearrange("p c (h w) -> p c h w", h=HP)
            nc.gpsimd.memset(r[:, :, 0, :], 0.0)
            nc.gpsimd.memset(r[:, :, HP - 1, :], 0.0)
            nc.gpsimd.memset(r[:, :, 1:HP - 1, 0:1], 0.0)
            nc.gpsimd.memset(r[:, :, 1:HP - 1, WP - 1:WP], 0.0)
            tiles.append(t)
        return tiles

    xpad1h = cpool.tile([128, 2, NPPAD], F8, tag="xpadh1")
    xpad1l = cpool.tile([128, 2, NPPAD], F8, tag="xpadl1")
    xpadh = [xpad0h, xpad1h]
    xpadl = [xpad0l, xpad1l]
    o1padh = make_pads("o1padh", 1)[0]
    o1padl = make_pads("o1padl", 1)[0]

    xl_r = xl8.rearrange("s (c p) h w -> s p c (h w)", c=2)

    def load_x_a(s):
        """direct DMA of pre-padded fp8 hi/lo images; gap accum via ACT."""
        j = s % 2
        gsum = []
        if s > 0:
            for c in range(2):
                nc.sync.dma_start(xpadh[j][:, c], xh_r[s, :, c])
        for c in range(2):
            g = small_p.tile([128, 1], F32, tag="gsum")
            nc.scalar.activation(accsink[:], xpadh[j][:, c],
                                 AF.Copy, accum_out=g[:])
            gsum.append(g)
        if s > 0:
            for c in range(2):
                nc.sync.dma_start(xpadl[j][:, c], xl_r[s, :, c])
        return gsum

    def load_x_b(s, gsum):
        gps = psmall_p.tile([16, 1], F32, tag="gap_ps")
        for c in range(2):
            nc.tensor.matmul(gps[:], rwT_sb[c][:], gsum[c][:],
                             start=(c == 0), stop=(c == 1))
        nc.scalar.activation(gap16[:, s:s + 1], gps[:], AF.Identity,
                             bias=rb_sb[:], scale=1.0)

    def gen_weights_a(wn, s):
        """a = sigmoid(fc1p(gap)); broadcast to aexp via DRAM roundtrip."""
        aps = psmall_p.tile([128, 32], F32, tag="avec_ps")
        for jj in range(32):
            nc.tensor.matmul(aps[:, jj:jj + 1],
                             fc1wT_sb[wn][:, 128 * jj:128 * (jj + 1)],
                             gap16[:, s:s + 1],
                             start=True, stop=True)
        avt = small_p.tile([128, 32], F32, tag="avtmp")
        nc.vector.tensor_add(avt[:], aps[:], fc1b_sb[wn][:])
        avec = small_p.tile([128, 32], BF16, tag="avec")
        nc.scalar.activation(avec[:], avt[:], AF.Sigmoid)
        avd = dram_p.tile([4096], BF16, tag="avd")
        nc.scalar.dma_start(avd[:].rearrange("(j p) -> p j", p=128), avec[:])
        # broadcast to [128, (ch, i, co)]: per 64-partition block one
        # contiguous 2048-element replicated read
        aexp = aexp_p.tile([128, 2 * 4 * 256], BF16, tag="aexp")
        avr = avd[:].rearrange("(hp f) -> hp f", hp=2)
        for hp in range(2):
            nc.scalar.dma_start(aexp[64 * hp:64 * (hp + 1), :],
                              avr[hp].unsqueeze(0).broadcast_to([64, 2048]))
        return aexp

    def gen_weights_b(wn, aexp):
        """W = sum_i a_i*w2_i (bf16), split into fp8 hi/lo."""
        ae4 = aexp[:].rearrange("p (c i o) -> p c i o", c=2, i=4)
        wv = lambda t: t[:].rearrange("p (c k o) -> p c k o", c=2, k=9)

        def abid(i):
            return (ae4[:, :, i, :].unsqueeze(2)
                    .broadcast_to([128, 2, 9, 256]))

        wbf = wbf_p.tile([128, 2 * 9 * 256], BF16, tag="wbf")
        nc.vector.tensor_mul(wv(wbf), wv(w2sb[wn][0]), abid(0))
        for i in range(1, 4):
            tmp = wtmp_p.tile([128, 2 * 9 * 256], BF16, tag="wtmp")
            nc.vector.tensor_mul(wv(tmp), wv(w2sb[wn][i]), abid(i))
            nc.vector.tensor_add(wbf[:], wbf[:], tmp[:])
        wh = w8_p.tile([128, 2 * 9 * 256], F8, tag="wh")
        nc.gpsimd.tensor_copy(wh[:], wbf[:])
        wl = w8_p.tile([128, 2 * 9 * 256], F8, tag="wl")
        nc.vector.tensor_sub(wl[:], wbf[:], wh[:])
        return wh, wl

    def conv(wh, wl, srch, srcl, sink):
        """3-pass fp8 DoubleRow 3x3 conv; sink(cc, t, psum_tile) evacuates."""
        whv = wh[:].rearrange("p (c k o) -> p c k o", c=2, k=9)
        wlv = wl[:].rearrange("p (c k o) -> p c k o", c=2, k=9)
        sh = srch[:].rearrange("p c (h w) -> p c h w", h=HP)
        sl = srcl[:].rearrange("p c (h w) -> p c h w", h=HP)
        for cc in range(2):
            for t in range(NT):
                ps = psum_p.tile([128, NFREE], F32, tag="cps")
                idx = 0
                for kg in range(3):
                    for wop, xop in ((whv, sh), (whv, sl), (wlv, sh)):
                        for kk in range(3):
                            k = 3 * kg + kk
                            kh, kw = divmod(k, 3)
                            r0 = TROWS * t + kh
                            nc.tensor.matmul(
                                ps[:],
                                wop[:, :, k, 128 * cc:128 * (cc + 1)],
                                xop[:, :, r0:r0 + TROWS, kw:kw + W],
                                start=(idx == 0), stop=(idx == 26),
                                perf_mode=DR)
                            idx += 1
                sink(cc, t, ps)

    # ---------------- prologue ----------------
    g0 = load_x_a(0)
    load_w2sb(0)
    load_bn_consts()
    load_x_b(0, g0)
    ax = gen_weights_a(0, 0)
    w1 = gen_weights_b(0, ax)
    load_deferred_consts()
    load_w2sb(1)
    ax = gen_weights_a(1, 0)
    w2 = gen_weights_b(1, ax)
    gsum_next = load_x_a(1)

    for s in range(BL):
        j = s % 2
        # generate next sample's weights one full iteration ahead
        if s + 1 < BL:
            load_x_b(s + 1, gsum_next)
            ax1 = gen_weights_a(0, s + 1)
            ax2 = gen_weights_a(1, s + 1)
            w1_next = gen_weights_b(0, ax1)

        # ---- conv1 + bn1(+*SO1) + relu -> o1 hi/lo (fp8, padded) ----
        oph = o1padh[:].rearrange("p c (h w) -> p c h w", h=HP)
        opl = o1padl[:].rearrange("p c (h w) -> p c h w", h=HP)

        def sink1(cc, t, ps):
            rows = slice(TROWS * t + 1, TROWS * t + 1 + TROWS)
            psv = ps[:].rearrange("p (h w) -> p h w", h=TROWS)
            nc.scalar.activation(oph[:, cc, rows, 1:1 + W], psv,
                                 AF.Relu, bias=bnb1_sb[cc][:],
                                 scale=bns_sb[0][cc][:])
            obf = stage_p.tile([128, TROWS, W], BF16, tag="o1bf")
            nc.scalar.activation(obf[:], psv, AF.Relu, bias=bnb1_sb[cc][:],
                                 scale=bns_sb[0][cc][:])
            nc.vector.tensor_sub(opl[:, cc, rows, 1:1 + W], obf[:],
                                 oph[:, cc, rows, 1:1 + W])

        conv(w1[0], w1[1], xpadh[j], xpadl[j], sink1)

        if s + 2 < BL:
            gsum_next = load_x_a(s + 2)
        if s + 1 < BL:
            w2_next = gen_weights_b(1, ax2)

        # ---- conv2 + bn2 + residual + relu -> out ----
        def sink2(cc, t, ps):
            t2 = stage_p.tile([128, NFREE], F32, tag="t2")
            nc.scalar.activation(t2[:], ps[:], AF.Identity,
                                 scale=bns_sb[1][cc][:])
            xres = stage_p.tile([128, NFREE], BF16, tag="xres")
            xflat = xb2[s, 128 * cc:128 * (cc + 1)].rearrange(
                "c h w -> c (h w)")
            nc.sync.dma_start(xres[:], xflat[:, NFREE * t:NFREE * (t + 1)])
            nc.gpsimd.tensor_add(t2[:], t2[:], xres[:])
            nc.gpsimd.tensor_scalar_max(t2[:], t2[:], 0.0)
            oflat = out4[s, 128 * cc:128 * (cc + 1)].rearrange(
                "c h w -> c (h w)")
            nc.sync.dma_start(oflat[:, NFREE * t:NFREE * (t + 1)], t2[:])

        conv(w2[0], w2[1], o1padh, o1padl, sink2)
        if s + 1 < BL:
            w1 = w1_next
            w2 = w2_next

    ctx.close()


_NC_CACHE = {}


def get_program():
    if "nc" not in _NC_CACHE:
        _NC_CACHE["nc"] = build_program()
    return _NC_CACHE["nc"]


def prep_inputs(inputs):
    f32 = lambda a: np.ascontiguousarray(np.asarray(a, np.float32))
    bf = lambda a: np.ascontiguousarray(
        np.asarray(a, np.float32).astype(BFNP))

    x = np.asarray(inputs["x"], np.float32)

    # fp8 hi/lo split of x*SX (exact residual, same scale); both pre-padded
    xs = x * SX
    xh = np.zeros((B, C, HP, WP), E4NP)
    xh[:, :, 1:1 + H, 1:1 + W] = xs.astype(E4NP)
    xl = np.zeros((B, C, HP, WP), E4NP)
    xl[:, :, 1:1 + H, 1:1 + W] = (xs - xh[:, :, 1:1 + H, 1:1 + W]
                                  .astype(np.float32)).astype(E4NP)

    def perm_fc1():
        n = np.arange(4096)
        return (16 * (n % 256) + 8 * ((n // 1024) % 2) + 4 * (n // 2048)
                + (n // 256) % 4)

    PI = perm_fc1()

    def pack_fc1(fc1_w, fc1_b):
        wT = np.asarray(fc1_w, np.float32).T      # [16, 4096]
        wp = bf(wT[:, PI])
        b = np.asarray(fc1_b, np.float32)[PI]     # [4096] permuted
        bp = f32(b.reshape(32, 128).T)            # [128, 32]
        return wp, bp

    def pack_w2(fc2_w):
        w2 = np.asarray(fc2_w, np.float32).reshape(1024, 576, 4) * SW
        p = np.arange(128)
        ch = np.arange(2)
        k = np.arange(9)
        co = np.arange(256)
        # [p, ch, k, co]
        g = (co[None, None, None, :] * 4 + 2 * ch[None, :, None, None]
             + (p[:, None, None, None] // 64))
        o = (p[:, None, None, None] % 64) * 9 + k[None, None, :, None]
        out = np.empty((4, 128, 2, 9, 256), np.float32)
        for i in range(4):
            out[i] = w2[g, o, i]
        return bf(out.reshape(4, 128, 2 * 9 * 256))

    def bn_fold(g, b, m, v):
        sc = np.asarray(g, np.float32) / np.sqrt(np.asarray(v, np.float32) + EPS)
        bia = np.asarray(b, np.float32) - np.asarray(m, np.float32) * sc
        return sc, bia

    sc1, bia1 = bn_fold(inputs["bn1_g"], inputs["bn1_b"], inputs["bn1_m"],
                        inputs["bn1_v"])
    sc2, bia2 = bn_fold(inputs["bn2_g"], inputs["bn2_b"], inputs["bn2_m"],
                        inputs["bn2_v"])

    fc1w1, fc1b1 = pack_fc1(inputs["w1_fc1_w"], inputs["w1_fc1_b"])
    fc1w2, fc1b2 = pack_fc1(inputs["w2_fc1_w"], inputs["w2_fc1_b"])

    base = {
        "rwT": f32((np.asarray(inputs["reduce_w"], np.float32).T
                    / (NPIX * SX)).reshape(2, 128, 16)),
        "rb": f32(np.asarray(inputs["reduce_b"]).reshape(16, 1)),
        "fc1wTp1": fc1w1, "fc1bp1": fc1b1,
        "fc1wTp2": fc1w2, "fc1bp2": fc1b2,
        "w2p1": pack_w2(inputs["w1_fc2_w"]),
        "w2p2": pack_w2(inputs["w2_fc2_w"]),
        "bns1": f32((sc1 * SO1 / (SX * SW)).reshape(2, 128, 1)),
        "bnb1": f32((bia1 * SO1).reshape(2, 128, 1)),
        "bns2": f32((sc2 / (SO1 * SW)).reshape(2, 128, 1)),
    }

    # residual with bn2 bias folded in
    xb2 = (x + bia2[None, :, None, None]).astype(BFNP)

    in_maps = []
    for i in range(NCORES):
        m = dict(base)
        sl = slice(i * BL, (i + 1) * BL)
        m["xh8p"] = np.ascontiguousarray(xh[sl])
        m["xl8p"] = np.ascontiguousarray(xl[sl])
        m["xb2"] = np.ascontiguousarray(xb2[sl])
        in_maps.append(m)
    return in_maps


def kernel(**inputs):
    in_maps = prep_inputs(inputs)
    nc = get_program()
    res = bass_utils.run_bass_kernel_spmd(nc, in_maps,
                                          core_ids=list(range(NCORES)))
    out = np.concatenate([r["out4"] for r in res.results], axis=0)
    return out.astype(np.float32)


# revision 25
# speedup vs baseline: 1.2856x; 1.0473x over previous
"""Trainium2 Bass kernel for nn_BasicBlock (WeightNet/CondConv-style block).

Data parallel over batch: 32 samples -> 8 cores x 4 samples.

fp8 (e4m3) 3-pass DoubleRow conv: every conv operand (x, o1, W1, W2) is split
into fp8 hi + lo (exact residual, same scale), and each 3x3 conv accumulates
  Wh@xh + Wh@xl + Wl@xh
in fp32 PSUM via DoubleRow matmuls (contraction 256 = 2x128 chunks per
instruction).  Dropping the lo*lo term keeps bf16-level accuracy at 27/36 of
the bf16 matmul row count.  Scales: x*16, W*64 (basis pre-scaled on host),
o1*2; all folded into the BN affine host-side, so all three passes accumulate
at a single PSUM scale.

Per core, per sample:
  gap   = sum(x_hi, HW) @ rwT + rb                    (ACT accum + PE)
  a     = sigmoid(fc1p(gap))  (fc1 host-permuted so the broadcast of a to
          [128, ch, i, co] is two contiguous stride-0-partition DMAs)
  W     = sum_i a_i * w2p_i   (DVE broadcast-mul chain, bf16)
  Wh/Wl = fp8 split            (Pool copy + DVE sub)
  conv1 -> bn1+relu -> o1 hi/lo (ACT x2 + DVE sub), conv2 -> bn2 fold
  out   = relu(bn2(conv2) + x) (ACT psum evac, Pool add+max)
"""

import sys

sys.path.insert(0, "/opt/trn_rl_repo")

import numpy as np
import ml_dtypes

import concourse.bass as bass
import concourse.tile as tile
from concourse import bacc, mybir
from concourse import bass_utils

F32 = mybir.dt.float32
BF16 = mybir.dt.bfloat16
F8 = mybir.dt.float8e4
AF = mybir.ActivationFunctionType
DR = mybir.MatmulPerfMode.DoubleRow
ALU = mybir.AluOpType

E4NP = ml_dtypes.float8_e4m3
BFNP = ml_dtypes.bfloat16

B, C, H, W = 32, 256, 56, 56
NCORES = 8
BL = B // NCORES          # samples per core
HP, WP = H + 2, W + 2     # padded 58x58
NPIX = H * W              # 3136
NPPAD = HP * WP           # 3364
NT = 7                    # h-tiles of 8 rows
TROWS = 8
NFREE = TROWS * W         # 448 columns per conv output tile
EPS = 1e-5
SX, SW, SO1 = 16.0, 64.0, 2.0


def build_program():
    nc = bacc.Bacc("TRN2", target_bir_lowering=False, debug=False,
                   num_devices=NCORES)

    xh8 = nc.dram_tensor("xh8p", [BL, C, HP, WP], F8, kind="ExternalInput").ap()
    xl8 = nc.dram_tensor("xl8p", [BL, C, HP, WP], F8, kind="ExternalInput").ap()
    xb2 = nc.dram_tensor("xb2", [BL, C, H, W], BF16, kind="ExternalInput").ap()
    out4 = nc.dram_tensor("out4", [BL, C, H, W], F32, kind="ExternalOutput").ap()
    rwT = nc.dram_tensor("rwT", [2, 128, 16], F32, kind="ExternalInput").ap()
    rb = nc.dram_tensor("rb", [16, 1], F32, kind="ExternalInput").ap()
    fc1wT = [nc.dram_tensor(f"fc1wTp{n}", [17, 4096], BF16,
                            kind="ExternalInput").ap() for n in (1, 2)]
    w2p = [nc.dram_tensor(f"w2p{n}", [4, 128, 2 * 9 * 256], BF16,
                          kind="ExternalInput").ap() for n in (1, 2)]
    bns = [nc.dram_tensor(f"bns{n}", [2, 128, 1], F32,
                          kind="ExternalInput").ap() for n in (1, 2)]
    bnb1 = nc.dram_tensor("bnb1", [2, 128, 1], F32, kind="ExternalInput").ap()

    with tile.TileContext(nc) as tc:
        build_body(tc, xh8, xl8, xb2, out4, rwT, rb, fc1wT, w2p, bns,
                   bnb1)

    nc.compile()
    return nc


def build_body(tc, xh8, xl8, xb2, out4, rwT, rb, fc1wT, w2p, bns, bnb1):
    nc = tc.nc
    from contextlib import ExitStack
    ctx = ExitStack()

    cpool = ctx.enter_context(tc.tile_pool(name="consts", bufs=1))
    wbf_p = ctx.enter_context(tc.tile_pool(name="wbf", bufs=1))
    wtmp_p = ctx.enter_context(tc.tile_pool(name="wtmp", bufs=1))
    w8_p = ctx.enter_context(tc.tile_pool(name="w8", bufs=4))
    aexp_p = ctx.enter_context(tc.tile_pool(name="aexp", bufs=2))
    small_p = ctx.enter_context(tc.tile_pool(name="small", bufs=4))
    stage_p = ctx.enter_context(tc.tile_pool(name="stage", bufs=3))
    psum_p = ctx.enter_context(tc.tile_pool(name="psum", bufs=6, space="PSUM"))
    psmall_p = ctx.enter_context(tc.tile_pool(name="psmall", bufs=2,
                                              space="PSUM"))
    dram_p = ctx.enter_context(tc.tile_pool(name="dscratch", bufs=2,
                                            space="DRAM"))

    xh_r = xh8.rearrange("s (c p) h w -> s p c (h w)", c=2)
    xl_r0 = xl8.rearrange("s (c p) h w -> s p c (h w)", c=2)

    # ACT table warmup: trigger the LoadActFuncSet at t~0, off the x path
    warm = cpool.tile([1, 1], F32, tag="warm")
    nc.gpsimd.memset(warm[:], 0.0)
    nc.scalar.activation(warm[:], warm[:], AF.Copy)

    # sample-0 image DMAs lead the queue: they head the gap->fc1->wgen chain
    xpad0h = cpool.tile([128, 2, NPPAD], F8, tag="xpadh0")
    xpad0l = cpool.tile([128, 2, NPPAD], F8, tag="xpadl0")
    for c in range(2):
        nc.sync.dma_start(xpad0h[:, c], xh_r[0, :, c])
    for c in range(2):
        nc.sync.dma_start(xpad0l[:, c], xl_r0[0, :, c])

    # ---- resident constants ----
    rwT_sb = []
    for c in range(2):
        t = cpool.tile([128, 16], F32, tag=f"rwT{c}")
        nc.sync.dma_start(t[:], rwT[c])
        rwT_sb.append(t)
    rb_sb = cpool.tile([16, 1], F32, tag="rb")
    nc.sync.dma_start(rb_sb[:], rb)

    fc1wT_sb = []
    for n in range(2):
        t = cpool.tile([17, 4096], BF16, tag=f"fc1wT{n}")
        if n == 0:
            nc.sync.dma_start(t[:], fc1wT[n])
        fc1wT_sb.append(t)

    w2sb = []   # [n][i] -> [128, 2*9*256] bf16, layout (ch, k, co)
    for n in range(2):
        per = []
        for i in range(4):
            t = cpool.tile([128, 2 * 9 * 256], BF16, tag=f"w2sb{n}{i}")
            per.append(t)
        w2sb.append(per)

    NWQ = 4   # chunks per basis-tile DMA: keeps DMA_ENGINES holds short
    def load_w2sb(n):
        q = 4608 // NWQ
        for i in range(4):
            for j in range(NWQ):
                nc.sync.dma_start(w2sb[n][i][:, q * j:q * (j + 1)],
                                  w2p[n][i][:, q * j:q * (j + 1)])

    bns_sb = []
    for n in range(2):
        per = []
        for cc in range(2):
            t = cpool.tile([128, 1], F32, tag=f"bns{n}{cc}")
            per.append(t)
        bns_sb.append(per)
    bnb1_sb = []
    for cc in range(2):
        t = cpool.tile([128, 1], F32, tag=f"bnb1{cc}")
        bnb1_sb.append(t)

    def load_bn_consts():
        for n in range(2):
            for cc in range(2):
                nc.sync.dma_start(bns_sb[n][cc][:], bns[n][cc])
        for cc in range(2):
            nc.sync.dma_start(bnb1_sb[cc][:], bnb1[cc])

    def load_deferred_consts():
        nc.sync.dma_start(fc1wT_sb[1][:], fc1wT[1])

    gap16 = cpool.tile([17, BL], BF16, tag="gap16")
    nc.gpsimd.memset(gap16[:], 1.0)
    accsink = cpool.tile([128, NPPAD], F8, tag="accsink")

    # persistent padded tiles (double-buffered by sample parity);
    # borders zeroed once here, only interiors are rewritten per sample.
    def make_pads(name, nbuf):
        tiles = []
        for j in range(nbuf):
            t = cpool.tile([128, 2, NPPAD], F8, tag=f"{name}{j}")
            r = t[:].rearrange("p c (h w) -> p c h w", h=HP)
            nc.gpsimd.memset(r[:, :, 0, :], 0.0)
            nc.gpsimd.memset(r[:, :, HP - 1, :], 0.0)
            nc.gpsimd.memset(r[:, :, 1:HP - 1, 0:1], 0.0)
            nc.gpsimd.memset(r[:, :, 1:HP - 1, WP - 1:WP], 0.0)
            tiles.append(t)
        return tiles

    xpad1h = cpool.tile([128, 2, NPPAD], F8, tag="xpadh1")
    xpad1l = cpool.tile([128, 2, NPPAD], F8, tag="xpadl1")
    xpadh = [xpad0h, xpad1h]
    xpadl = [xpad0l, xpad1l]
    o1padh = make_pads("o1padh", 1)[0]
    o1padl = make_pads("o1padl", 1)[0]

    xl_r = xl8.rearrange("s (c p) h w -> s p c (h w)", c=2)

    def load_x_a(s):
        """direct DMA of pre-padded fp8 hi/lo images; gap sums via DVE."""
        j = s % 2
        gsum = []
        if s > 0:
            for c in range(2):
                nc.sync.dma_start(xpadh[j][:, c], xh_r[s, :, c])
        for c in range(2):
            g = small_p.tile([128, 1], F32, tag="gsum")
            if s < 2:
                # prologue: keep DVE free for the first weight chains
                nc.scalar.activation(accsink[:], xpadh[j][:, c],
                                     AF.Copy, accum_out=g[:])
            else:
                nc.vector.tensor_reduce(g[:], xpadh[j][:, c],
                                        mybir.AxisListType.X, ALU.add)
            gsum.append(g)
        if s > 0:
            for c in range(2):
                nc.sync.dma_start(xpadl[j][:, c], xl_r[s, :, c])
        return gsum

    def load_x_b(s, gsum):
        gpt = psmall_p.tile([128, 33], F32, tag="avec_ps")
        gps = gpt[0:16, 32:33]
        for c in range(2):
            nc.tensor.matmul(gps, rwT_sb[c][:], gsum[c][:],
                             start=(c == 0), stop=(c == 1))
        nc.scalar.activation(gap16[0:16, s:s + 1], gps, AF.Identity,
                             bias=rb_sb[:], scale=1.0)

    def gen_weights_a(wn, s):
        """a = sigmoid(fc1p(gap)); broadcast to aexp via DRAM roundtrip."""
        apt = psmall_p.tile([128, 33], F32, tag="avec_ps")
        aps = apt[:, 0:32]
        for jj in range(32):
            nc.tensor.matmul(aps[:, jj:jj + 1],
                             fc1wT_sb[wn][:, 128 * jj:128 * (jj + 1)],
                             gap16[:, s:s + 1],
                             start=True, stop=True)
        avec = small_p.tile([128, 32], BF16, tag="avec")
        nc.scalar.activation(avec[:], aps, AF.Sigmoid)
        avd = dram_p.tile([4096], BF16, tag="avd")
        nc.scalar.dma_start(avd[:].rearrange("(j p) -> p j", p=128), avec[:])
        # broadcast to [128, (ch, i, co)]: per 64-partition block one
        # contiguous 2048-element replicated read
        aexp = aexp_p.tile([128, 2 * 4 * 256], BF16, tag="aexp")
        avr = avd[:].rearrange("(hp f) -> hp f", hp=2)
        for hp in range(2):
            nc.scalar.dma_start(aexp[64 * hp:64 * (hp + 1), :],
                              avr[hp].unsqueeze(0).broadcast_to([64, 2048]))
        return aexp

    def gen_weights_b(wn, aexp, fast=False):
        """W = sum_i a_i*w2_i (bf16), split into fp8 hi/lo."""
        ae4 = aexp[:].rearrange("p (c i o) -> p c i o", c=2, i=4)
        wv = lambda t: t[:].rearrange("p (c k o) -> p c k o", c=2, k=9)

        def abid(i):
            return (ae4[:, :, i, :].unsqueeze(2)
                    .broadcast_to([128, 2, 9, 256]))

        wbf = wbf_p.tile([128, 2 * 9 * 256], BF16, tag="wbf")
        nc.vector.tensor_mul(wv(wbf), wv(w2sb[wn][0]), abid(0))
        for i in range(1, 4):
            tmp = wtmp_p.tile([128, 2 * 9 * 256], BF16, tag="wtmp")
            nc.vector.tensor_mul(wv(tmp), wv(w2sb[wn][i]), abid(i))
            nc.vector.tensor_add(wbf[:], wbf[:], tmp[:])
        wh = w8_p.tile([128, 2 * 9 * 256], F8, tag="wh")
        heng = nc.vector if fast else nc.gpsimd
        heng.tensor_copy(wh[:], wbf[:])
        wl = w8_p.tile([128, 2 * 9 * 256], F8, tag="wl")
        nc.vector.tensor_sub(wl[:], wbf[:], wh[:])
        return wh, wl

    def conv(wh, wl, srch, srcl, sink):
        """3-pass fp8 DoubleRow 3x3 conv; sink(cc, t, psum_tile) evacuates."""
        whv = wh[:].rearrange("p (c k o) -> p c k o", c=2, k=9)
        wlv = wl[:].rearrange("p (c k o) -> p c k o", c=2, k=9)
        sh = srch[:].rearrange("p c (h w) -> p c h w", h=HP)
        sl = srcl[:].rearrange("p c (h w) -> p c h w", h=HP)
        for cc in range(2):
            for t in range(NT):
                ps = psum_p.tile([128, NFREE], F32, tag="cps")
                idx = 0
                for kg in range(3):
                    for wop, xop in ((whv, sh), (whv, sl), (wlv, sh)):
                        for kk in range(3):
                            k = 3 * kg + kk
                            kh, kw = divmod(k, 3)
                            r0 = TROWS * t + kh
                            nc.tensor.matmul(
                                ps[:],
                                wop[:, :, k, 128 * cc:128 * (cc + 1)],
                                xop[:, :, r0:r0 + TROWS, kw:kw + W],
                                start=(idx == 0), stop=(idx == 26),
                                perf_mode=DR)
                            idx += 1
                sink(cc, t, ps)

    # ---------------- prologue ----------------
    g0 = load_x_a(0)
    load_w2sb(0)
    load_bn_consts()
    load_x_b(0, g0)
    ax = gen_weights_a(0, 0)
    w1 = gen_weights_b(0, ax)
    gsum_next = load_x_a(1)
    load_deferred_consts()
    load_w2sb(1)
    ax = gen_weights_a(1, 0)
    w2 = gen_weights_b(1, ax)

    for s in range(BL):
        j = s % 2
        # generate next sample's weights one full iteration ahead
        if s + 1 < BL:
            load_x_b(s + 1, gsum_next)
            ax1 = gen_weights_a(0, s + 1)
            ax2 = gen_weights_a(1, s + 1)
            w1_next = gen_weights_b(0, ax1)

        # ---- conv1 + bn1(+*SO1) + relu -> o1 hi/lo (fp8, padded) ----
        oph = o1padh[:].rearrange("p c (h w) -> p c h w", h=HP)
        opl = o1padl[:].rearrange("p c (h w) -> p c h w", h=HP)

        def sink1(cc, t, ps):
            rows = slice(TROWS * t + 1, TROWS * t + 1 + TROWS)
            psv = ps[:].rearrange("p (h w) -> p h w", h=TROWS)
            nc.scalar.activation(oph[:, cc, rows, 1:1 + W], psv,
                                 AF.Relu, bias=bnb1_sb[cc][:],
                                 scale=bns_sb[0][cc][:])
            obf = stage_p.tile([128, TROWS, W], BF16, tag="o1bf")
            nc.scalar.activation(obf[:], psv, AF.Relu, bias=bnb1_sb[cc][:],
                                 scale=bns_sb[0][cc][:])
            nc.vector.tensor_sub(opl[:, cc, rows, 1:1 + W], obf[:],
                                 oph[:, cc, rows, 1:1 + W])

        conv(w1[0], w1[1], xpadh[j], xpadl[j], sink1)

        if s + 2 < BL:
            gsum_next = load_x_a(s + 2)
        if s + 1 < BL:
            w2_next = gen_weights_b(1, ax2)

        # ---- conv2 + bn2 + residual + relu -> out ----
        def sink2(cc, t, ps):
            t2 = stage_p.tile([128, NFREE], F32, tag="t2")
            nc.scalar.activation(t2[:], ps[:], AF.Identity,
                                 scale=bns_sb[1][cc][:])
            xres = stage_p.tile([128, NFREE], BF16, tag="xres")
            xflat = xb2[s, 128 * cc:128 * (cc + 1)].rearrange(
                "c h w -> c (h w)")
            nc.sync.dma_start(xres[:], xflat[:, NFREE * t:NFREE * (t + 1)])
            eng = nc.vector if s == BL - 1 else nc.gpsimd
            eng.tensor_add(t2[:], t2[:], xres[:])
            eng.tensor_scalar_max(t2[:], t2[:], 0.0)
            oflat = out4[s, 128 * cc:128 * (cc + 1)].rearrange(
                "c h w -> c (h w)")
            nc.sync.dma_start(oflat[:, NFREE * t:NFREE * (t + 1)], t2[:])

        conv(w2[0], w2[1], o1padh, o1padl, sink2)
        if s + 1 < BL:
            w1 = w1_next
            w2 = w2_next

    ctx.close()


_NC_CACHE = {}


def get_program():
    if "nc" not in _NC_CACHE:
        _NC_CACHE["nc"] = build_program()
    return _NC_CACHE["nc"]


def prep_inputs(inputs):
    f32 = lambda a: np.ascontiguousarray(np.asarray(a, np.float32))
    bf = lambda a: np.ascontiguousarray(
        np.asarray(a, np.float32).astype(BFNP))

    x = np.asarray(inputs["x"], np.float32)

    # fp8 hi/lo split of x*SX (exact residual, same scale); both pre-padded
    xs = x * SX
    xh = np.zeros((B, C, HP, WP), E4NP)
    xh[:, :, 1:1 + H, 1:1 + W] = xs.astype(E4NP)
    xl = np.zeros((B, C, HP, WP), E4NP)
    xl[:, :, 1:1 + H, 1:1 + W] = (xs - xh[:, :, 1:1 + H, 1:1 + W]
                                  .astype(np.float32)).astype(E4NP)

    def perm_fc1():
        n = np.arange(4096)
        return (16 * (n % 256) + 8 * ((n // 1024) % 2) + 4 * (n // 2048)
                + (n // 256) % 4)

    PI = perm_fc1()

    def pack_fc1(fc1_w, fc1_b):
        wT = np.asarray(fc1_w, np.float32).T      # [16, 4096]
        aug = np.concatenate([wT, np.asarray(fc1_b, np.float32)[None, :]],
                             axis=0)              # [17, 4096]
        return bf(aug[:, PI])

    def pack_w2(fc2_w):
        w2 = np.asarray(fc2_w, np.float32).reshape(1024, 576, 4) * SW
        p = np.arange(128)
        ch = np.arange(2)
        k = np.arange(9)
        co = np.arange(256)
        # [p, ch, k, co]
        g = (co[None, None, None, :] * 4 + 2 * ch[None, :, None, None]
             + (p[:, None, None, None] // 64))
        o = (p[:, None, None, None] % 64) * 9 + k[None, None, :, None]
        out = np.empty((4, 128, 2, 9, 256), np.float32)
        for i in range(4):
            out[i] = w2[g, o, i]
        return bf(out.reshape(4, 128, 2 * 9 * 256))

    def bn_fold(g, b, m, v):
        sc = np.asarray(g, np.float32) / np.sqrt(np.asarray(v, np.float32) + EPS)
        bia = np.asarray(b, np.float32) - np.asarray(m, np.float32) * sc
        return sc, bia

    sc1, bia1 = bn_fold(inputs["bn1_g"], inputs["bn1_b"], inputs["bn1_m"],
                        inputs["bn1_v"])
    sc2, bia2 = bn_fold(inputs["bn2_g"], inputs["bn2_b"], inputs["bn2_m"],
                        inputs["bn2_v"])

    fc1w1 = pack_fc1(inputs["w1_fc1_w"], inputs["w1_fc1_b"])
    fc1w2 = pack_fc1(inputs["w2_fc1_w"], inputs["w2_fc1_b"])

    base = {
        "rwT": f32((np.asarray(inputs["reduce_w"], np.float32).T
                    / (NPIX * SX)).reshape(2, 128, 16)),
        "rb": f32(np.asarray(inputs["reduce_b"]).reshape(16, 1)),
        "fc1wTp1": fc1w1, "fc1wTp2": fc1w2,
        "w2p1": pack_w2(inputs["w1_fc2_w"]),
        "w2p2": pack_w2(inputs["w2_fc2_w"]),
        "bns1": f32((sc1 * SO1 / (SX * SW)).reshape(2, 128, 1)),
        "bnb1": f32((bia1 * SO1).reshape(2, 128, 1)),
        "bns2": f32((sc2 / (SO1 * SW)).reshape(2, 128, 1)),
    }

    # residual with bn2 bias folded in
    xb2 = (x + bia2[None, :, None, None]).astype(BFNP)

    in_maps = []
    for i in range(NCORES):
        m = dict(base)
        sl = slice(i * BL, (i + 1) * BL)
        m["xh8p"] = np.ascontiguousarray(xh[sl])
        m["xl8p"] = np.ascontiguousarray(xl[sl])
        m["xb2"] = np.ascontiguousarray(xb2[sl])
        in_maps.append(m)
    return in_maps


def kernel(**inputs):
    in_maps = prep_inputs(inputs)
    nc = get_program()
    res = bass_utils.run_bass_kernel_spmd(nc, in_maps,
                                          core_ids=list(range(NCORES)))
    out = np.concatenate([r["out4"] for r in res.results], axis=0)
    return out.astype(np.float32)


# revision 31
# speedup vs baseline: 1.3176x; 1.0249x over previous
"""Trainium2 Bass kernel for nn_BasicBlock (WeightNet/CondConv-style block).

Data parallel over batch: 32 samples -> 8 cores x 4 samples.

fp8 (e4m3) 3-pass DoubleRow conv: every conv operand (x, o1, W1, W2) is split
into fp8 hi + lo (exact residual, same scale), and each 3x3 conv accumulates
  Wh@xh + Wh@xl + Wl@xh
in fp32 PSUM via DoubleRow matmuls (contraction 256 = 2x128 chunks per
instruction).  Dropping the lo*lo term keeps bf16-level accuracy at 27/36 of
the bf16 matmul row count.  Scales: x*16, W*64 (basis pre-scaled on host),
o1*2; all folded into the BN affine host-side, so all three passes accumulate
at a single PSUM scale.

Per core, per sample:
  gap   = sum(x_hi, HW) @ rwT + rb                    (ACT accum + PE)
  a     = sigmoid(fc1p(gap))  (fc1 host-permuted so the broadcast of a to
          [128, ch, i, co] is two contiguous stride-0-partition DMAs)
  W     = sum_i a_i * w2p_i   (DVE broadcast-mul chain, bf16)
  Wh/Wl = fp8 split            (Pool copy + DVE sub)
  conv1 -> bn1+relu -> o1 hi/lo (ACT x2 + DVE sub), conv2 -> bn2 fold
  out   = relu(bn2(conv2) + x) (ACT psum evac, Pool add+max)
"""

import sys

sys.path.insert(0, "/opt/trn_rl_repo")

import numpy as np
import ml_dtypes

import concourse.bass as bass
import concourse.tile as tile
from concourse import bacc, mybir
from concourse import bass_utils

F32 = mybir.dt.float32
BF16 = mybir.dt.bfloat16
F8 = mybir.dt.float8e4
AF = mybir.ActivationFunctionType
DR = mybir.MatmulPerfMode.DoubleRow
ALU = mybir.AluOpType

E4NP = ml_dtypes.float8_e4m3
BFNP = ml_dtypes.bfloat16

B, C, H, W = 32, 256, 56, 56
NCORES = 8
BL = B // NCORES          # samples per core
HP, WP = H + 2, W + 2     # padded 58x58
NPIX = H * W              # 3136
NPPAD = HP * WP           # 3364
NT = 7                    # h-tiles of 8 rows
TROWS = 8
NFREE = TROWS * W         # 448 columns per conv output tile
EPS = 1e-5
SX, SW, SO1 = 16.0, 64.0, 2.0


def build_program():
    nc = bacc.Bacc("TRN2", target_bir_lowering=False, debug=False,
                   num_devices=NCORES)

    xh8 = nc.dram_tensor("xh8p", [BL, C, HP, WP], F8, kind="ExternalInput").ap()
    xl8 = nc.dram_tensor("xl8p", [BL, C, HP, WP], F8, kind="ExternalInput").ap()
    xb2 = nc.dram_tensor("xb2", [BL, C, H, W], BF16, kind="ExternalInput").ap()
    out4 = nc.dram_tensor("out4", [BL, C, H, W], F32, kind="ExternalOutput").ap()
    rwT = nc.dram_tensor("rwT", [2, 128, 16], F32, kind="ExternalInput").ap()
    rb = nc.dram_tensor("rb", [16, 1], F32, kind="ExternalInput").ap()
    fc1wT = [nc.dram_tensor(f"fc1wTp{n}", [17, 4096], BF16,
                            kind="ExternalInput").ap() for n in (1, 2)]
    w2p = [nc.dram_tensor(f"w2p{n}", [4, 128, 2 * 9 * 256], BF16,
                          kind="ExternalInput").ap() for n in (1, 2)]
    bns = [nc.dram_tensor(f"bns{n}", [2, 128, 1], F32,
                          kind="ExternalInput").ap() for n in (1, 2)]
    bnb1 = nc.dram_tensor("bnb1", [2, 128, 1], F32, kind="ExternalInput").ap()

    with tile.TileContext(nc) as tc:
        build_body(tc, xh8, xl8, xb2, out4, rwT, rb, fc1wT, w2p, bns,
                   bnb1)

    nc.compile()
    return nc


def build_body(tc, xh8, xl8, xb2, out4, rwT, rb, fc1wT, w2p, bns, bnb1):
    nc = tc.nc
    from contextlib import ExitStack
    ctx = ExitStack()

    cpool = ctx.enter_context(tc.tile_pool(name="consts", bufs=1))
    wbf_p = ctx.enter_context(tc.tile_pool(name="wbf", bufs=1))
    wtmp_p = ctx.enter_context(tc.tile_pool(name="wtmp", bufs=1))
    w8_p = ctx.enter_context(tc.tile_pool(name="w8", bufs=4))
    aexp_p = ctx.enter_context(tc.tile_pool(name="aexp", bufs=2))
    small_p = ctx.enter_context(tc.tile_pool(name="small", bufs=4))
    stage_p = ctx.enter_context(tc.tile_pool(name="stage", bufs=3))
    psum_p = ctx.enter_context(tc.tile_pool(name="psum", bufs=6, space="PSUM"))
    psmall_p = ctx.enter_context(tc.tile_pool(name="psmall", bufs=2,
                                              space="PSUM"))
    dram_p = ctx.enter_context(tc.tile_pool(name="dscratch", bufs=2,
                                            space="DRAM"))

    xh_r = xh8.rearrange("s (c p) h w -> s p c (h w)", c=2)
    xl_r0 = xl8.rearrange("s (c p) h w -> s p c (h w)", c=2)

    # ACT table warmup: trigger every LoadActFuncSet at t~0, off the x path
    warm = cpool.tile([1, 1], F32, tag="warm")
    nc.gpsimd.memset(warm[:], 0.0)
    for fn in (AF.Copy, AF.Identity, AF.Relu, AF.Sigmoid):
        nc.scalar.activation(warm[:], warm[:], fn)

    # sample-0 image DMAs lead the queue: they head the gap->fc1->wgen chain
    xpad0h = cpool.tile([128, 2, NPPAD], F8, tag="xpadh0")
    xpad0l = cpool.tile([128, 2, NPPAD], F8, tag="xpadl0")
    for c in range(2):
        nc.sync.dma_start(xpad0h[:, c], xh_r[0, :, c])
    for c in range(2):
        nc.sync.dma_start(xpad0l[:, c], xl_r0[0, :, c])

    # ---- resident constants ----
    rwT_sb = []
    for c in range(2):
        t = cpool.tile([128, 16], F32, tag=f"rwT{c}")
        nc.sync.dma_start(t[:], rwT[c])
        rwT_sb.append(t)
    rb_sb = cpool.tile([16, 1], F32, tag="rb")
    nc.sync.dma_start(rb_sb[:], rb)

    fc1wT_sb = []
    for n in range(2):
        t = cpool.tile([17, 4096], BF16, tag=f"fc1wT{n}")
        if n == 0:
            nc.sync.dma_start(t[:], fc1wT[n])
        fc1wT_sb.append(t)

    w2sb = []   # [n][i] -> [128, 2*9*256] bf16, layout (ch, k, co)
    for n in range(2):
        per = []
        for i in range(4):
            t = cpool.tile([128, 2 * 9 * 256], BF16, tag=f"w2sb{n}{i}")
            per.append(t)
        w2sb.append(per)

    NWQ = 4   # chunks per basis-tile DMA: keeps DMA_ENGINES holds short
    def load_w2sb(n):
        q = 4608 // NWQ
        for i in range(4):
            for j in range(NWQ):
                nc.sync.dma_start(w2sb[n][i][:, q * j:q * (j + 1)],
                                  w2p[n][i][:, q * j:q * (j + 1)])

    bns_sb = []
    for n in range(2):
        per = []
        for cc in range(2):
            t = cpool.tile([128, 1], F32, tag=f"bns{n}{cc}")
            per.append(t)
        bns_sb.append(per)
    bnb1_sb = []
    for cc in range(2):
        t = cpool.tile([128, 1], F32, tag=f"bnb1{cc}")
        bnb1_sb.append(t)

    def load_bn_consts():
        for n in range(2):
            for cc in range(2):
                nc.sync.dma_start(bns_sb[n][cc][:], bns[n][cc])
        for cc in range(2):
            nc.sync.dma_start(bnb1_sb[cc][:], bnb1[cc])

    def load_deferred_consts():
        nc.sync.dma_start(fc1wT_sb[1][:], fc1wT[1])

    gap16 = cpool.tile([17, BL], BF16, tag="gap16")
    nc.gpsimd.memset(gap16[:], 1.0)
    accsink = cpool.tile([128, NPPAD], F8, tag="accsink")

    # persistent padded tiles (double-buffered by sample parity);
    # borders zeroed once here, only interiors are rewritten per sample.
    def make_pads(name, nbuf):
        tiles = []
        for j in range(nbuf):
            t = cpool.tile([128, 2, NPPAD], F8, tag=f"{name}{j}")
            r = t[:].rearrange("p c (h w) -> p c h w", h=HP)
            nc.gpsimd.memset(r[:, :, 0, :], 0.0)
            nc.gpsimd.memset(r[:, :, HP - 1, :], 0.0)
            nc.gpsimd.memset(r[:, :, 1:HP - 1, 0:1], 0.0)
            nc.gpsimd.memset(r[:, :, 1:HP - 1, WP - 1:WP], 0.0)
            tiles.append(t)
        return tiles

    xpad1h = cpool.tile([128, 2, NPPAD], F8, tag="xpadh1")
    xpad1l = cpool.tile([128, 2, NPPAD], F8, tag="xpadl1")
    xpadh = [xpad0h, xpad1h]
    xpadl = [xpad0l, xpad1l]
    o1padh = make_pads("o1padh", 1)[0]
    o1padl = make_pads("o1padl", 1)[0]

    xl_r = xl8.rearrange("s (c p) h w -> s p c (h w)", c=2)

    def load_x_a(s):
        """direct DMA of pre-padded fp8 hi/lo images; gap sums via DVE."""
        j = s % 2
        gsum = []
        if s > 0:
            for c in range(2):
                nc.sync.dma_start(xpadh[j][:, c], xh_r[s, :, c])
        for c in range(2):
            g = small_p.tile([128, 1], F32, tag="gsum")
            if s < 2:
                # prologue: keep DVE free for the first weight chains
                nc.scalar.activation(accsink[:], xpadh[j][:, c],
                                     AF.Copy, accum_out=g[:])
            else:
                nc.vector.tensor_reduce(g[:], xpadh[j][:, c],
                                        mybir.AxisListType.X, ALU.add)
            gsum.append(g)
        if s > 0:
            for c in range(2):
                nc.sync.dma_start(xpadl[j][:, c], xl_r[s, :, c])
        return gsum

    def load_x_b(s, gsum):
        gpt = psmall_p.tile([128, 33], F32, tag="avec_ps")
        gps = gpt[0:16, 32:33]
        ng = len(gsum)
        for c in range(ng):
            nc.tensor.matmul(gps, rwT_sb[c * 2 // ng][:], gsum[c][:],
                             start=(c == 0), stop=(c == ng - 1))
        nc.scalar.activation(gap16[0:16, s:s + 1], gps, AF.Identity,
                             bias=rb_sb[:], scale=1.0)

    def gen_weights_a(wn, s):
        """a = sigmoid(fc1p(gap)); broadcast to aexp via DRAM roundtrip."""
        apt = psmall_p.tile([128, 33], F32, tag="avec_ps")
        aps = apt[:, 0:32]
        for jj in range(32):
            nc.tensor.matmul(aps[:, jj:jj + 1],
                             fc1wT_sb[wn][:, 128 * jj:128 * (jj + 1)],
                             gap16[:, s:s + 1],
                             start=True, stop=True)
        avec = small_p.tile([128, 32], BF16, tag="avec")
        nc.scalar.activation(avec[:], aps, AF.Sigmoid)
        avd = dram_p.tile([4096], BF16, tag="avd")
        # broadcast to [128, (ch, i, co)]: per 64-partition block one
        # contiguous 2048-element replicated read; write/read halves are
        # pipelined (aexp half hp depends only on avd half hp)
        aexp = aexp_p.tile([128, 2 * 4 * 256], BF16, tag="aexp")
        avr = avd[:].rearrange("(hp f) -> hp f", hp=2)
        for hp in range(2):
            nc.scalar.dma_start(
                avr[hp].rearrange("(j p) -> p j", p=128),
                avec[:, 16 * hp:16 * (hp + 1)])
            nc.scalar.dma_start(aexp[64 * hp:64 * (hp + 1), :],
                              avr[hp].unsqueeze(0).broadcast_to([64, 2048]))
        return aexp

    def gen_weights_b(wn, aexp, fast=False):
        """W = sum_i a_i*w2_i (bf16), split into fp8 hi/lo."""
        ae4 = aexp[:].rearrange("p (c i o) -> p c i o", c=2, i=4)
        wv = lambda t: t[:].rearrange("p (c k o) -> p c k o", c=2, k=9)

        def abid(i):
            return (ae4[:, :, i, :].unsqueeze(2)
                    .broadcast_to([128, 2, 9, 256]))

        wbf = wbf_p.tile([128, 2 * 9 * 256], BF16, tag="wbf")
        nc.vector.tensor_mul(wv(wbf), wv(w2sb[wn][0]), abid(0))
        for i in range(1, 4):
            tmp = wtmp_p.tile([128, 2 * 9 * 256], BF16, tag="wtmp")
            nc.vector.tensor_mul(wv(tmp), wv(w2sb[wn][i]), abid(i))
            nc.vector.tensor_add(wbf[:], wbf[:], tmp[:])
        wh = w8_p.tile([128, 2 * 9 * 256], F8, tag="wh")
        heng = nc.vector if fast else nc.gpsimd
        heng.tensor_copy(wh[:], wbf[:])
        wl = w8_p.tile([128, 2 * 9 * 256], F8, tag="wl")
        nc.vector.tensor_sub(wl[:], wbf[:], wh[:])
        return wh, wl

    def conv(wh, wl, srch, srcl, sink):
        """3-pass fp8 DoubleRow 3x3 conv; sink(cc, t, psum_tile) evacuates."""
        whv = wh[:].rearrange("p (c k o) -> p c k o", c=2, k=9)
        wlv = wl[:].rearrange("p (c k o) -> p c k o", c=2, k=9)
        sh = srch[:].rearrange("p c (h w) -> p c h w", h=HP)
        sl = srcl[:].rearrange("p c (h w) -> p c h w", h=HP)
        for cc in range(2):
            for t in range(NT):
                ps = psum_p.tile([128, NFREE], F32, tag="cps")
                idx = 0
                for kg in range(3):
                    for wop, xop in ((whv, sh), (whv, sl), (wlv, sh)):
                        for kk in range(3):
                            k = 3 * kg + kk
                            kh, kw = divmod(k, 3)
                            r0 = TROWS * t + kh
                            nc.tensor.matmul(
                                ps[:],
                                wop[:, :, k, 128 * cc:128 * (cc + 1)],
                                xop[:, :, r0:r0 + TROWS, kw:kw + W],
                                start=(idx == 0), stop=(idx == 26),
                                perf_mode=DR)
                            idx += 1
                sink(cc, t, ps)

    # ---------------- prologue ----------------
    g0 = load_x_a(0)
    load_w2sb(0)
    load_x_b(0, g0)
    ax = gen_weights_a(0, 0)
    load_bn_consts()
    w1 = gen_weights_b(0, ax, fast=True)
    gsum_next = load_x_a(1)
    load_deferred_consts()
    load_w2sb(1)
    ax = gen_weights_a(1, 0)
    w2 = gen_weights_b(1, ax)

    for s in range(BL):
        j = s % 2
        # generate next sample's weights one full iteration ahead
        if s + 1 < BL:
            load_x_b(s + 1, gsum_next)
            ax1 = gen_weights_a(0, s + 1)
            ax2 = gen_weights_a(1, s + 1)
            w1_next = gen_weights_b(0, ax1)

        # ---- conv1 + bn1(+*SO1) + relu -> o1 hi/lo (fp8, padded) ----
        oph = o1padh[:].rearrange("p c (h w) -> p c h w", h=HP)
        opl = o1padl[:].rearrange("p c (h w) -> p c h w", h=HP)

        def sink1(cc, t, ps):
            rows = slice(TROWS * t + 1, TROWS * t + 1 + TROWS)
            psv = ps[:].rearrange("p (h w) -> p h w", h=TROWS)
            nc.scalar.activation(oph[:, cc, rows, 1:1 + W], psv,
                                 AF.Relu, bias=bnb1_sb[cc][:],
                                 scale=bns_sb[0][cc][:])
            obf = stage_p.tile([128, TROWS, W], BF16, tag="o1bf")
            nc.scalar.activation(obf[:], psv, AF.Relu, bias=bnb1_sb[cc][:],
                                 scale=bns_sb[0][cc][:])
            nc.vector.tensor_sub(opl[:, cc, rows, 1:1 + W], obf[:],
                                 oph[:, cc, rows, 1:1 + W])

        conv(w1[0], w1[1], xpadh[j], xpadl[j], sink1)

        if s + 2 < BL:
            gsum_next = load_x_a(s + 2)
        if s + 1 < BL:
            w2_next = gen_weights_b(1, ax2)

        # ---- conv2 + bn2 + residual + relu -> out ----
        def sink2(cc, t, ps):
            t2 = stage_p.tile([128, NFREE], F32, tag="t2")
            nc.scalar.activation(t2[:], ps[:], AF.Identity,
                                 scale=bns_sb[1][cc][:])
            xres = stage_p.tile([128, NFREE], BF16, tag="xres")
            xflat = xb2[s, 128 * cc:128 * (cc + 1)].rearrange(
                "c h w -> c (h w)")
            nc.sync.dma_start(xres[:], xflat[:, NFREE * t:NFREE * (t + 1)])
            eng = nc.vector if s == BL - 1 else nc.gpsimd
            eng.tensor_add(t2[:], t2[:], xres[:])
            eng.tensor_scalar_max(t2[:], t2[:], 0.0)
            oflat = out4[s, 128 * cc:128 * (cc + 1)].rearrange(
                "c h w -> c (h w)")
            nc.sync.dma_start(oflat[:, NFREE * t:NFREE * (t + 1)], t2[:])

        conv(w2[0], w2[1], o1padh, o1padl, sink2)
        if s + 1 < BL:
            w1 = w1_next
            w2 = w2_next

    ctx.close()


_NC_CACHE = {}


def get_program():
    if "nc" not in _NC_CACHE:
        _NC_CACHE["nc"] = build_program()
    return _NC_CACHE["nc"]


def prep_inputs(inputs):
    f32 = lambda a: np.ascontiguousarray(np.asarray(a, np.float32))
    bf = lambda a: np.ascontiguousarray(
        np.asarray(a, np.float32).astype(BFNP))

    x = np.asarray(inputs["x"], np.float32)

    # fp8 hi/lo split of x*SX (exact residual, same scale); both pre-padded
    xs = x * SX
    xh = np.zeros((B, C, HP, WP), E4NP)
    xh[:, :, 1:1 + H, 1:1 + W] = xs.astype(E4NP)
    xl = np.zeros((B, C, HP, WP), E4NP)
    xl[:, :, 1:1 + H, 1:1 + W] = (xs - xh[:, :, 1:1 + H, 1:1 + W]
                                  .astype(np.float32)).astype(E4NP)

    def perm_fc1():
        n = np.arange(4096)
        return (16 * (n % 256) + 8 * ((n // 1024) % 2) + 4 * (n // 2048)
                + (n // 256) % 4)

    PI = perm_fc1()

    def pack_fc1(fc1_w, fc1_b):
        wT = np.asarray(fc1_w, np.float32).T      # [16, 4096]
        aug = np.concatenate([wT, np.asarray(fc1_b, np.float32)[None, :]],
                             axis=0)              # [17, 4096]
        return bf(aug[:, PI])

    def pack_w2(fc2_w):
        w2 = np.asarray(fc2_w, np.float32).reshape(1024, 576, 4) * SW
        p = np.arange(128)
        ch = np.arange(2)
        k = np.arange(9)
        co = np.arange(256)
        # [p, ch, k, co]
        g = (co[None, None, None, :] * 4 + 2 * ch[None, :, None, None]
             + (p[:, None, None, None] // 64))
        o = (p[:, None, None, None] % 64) * 9 + k[None, None, :, None]
        out = np.empty((4, 128, 2, 9, 256), np.float32)
        for i in range(4):
            out[i] = w2[g, o, i]
        return bf(out.reshape(4, 128, 2 * 9 * 256))

    def bn_fold(g, b, m, v):
        sc = np.asarray(g, np.float32) / np.sqrt(np.asarray(v, np.float32) + EPS)
        bia = np.asarray(b, np.float32) - np.asarray(m, np.float32) * sc
        return sc, bia

    sc1, bia1 = bn_fold(inputs["bn1_g"], inputs["bn1_b"], inputs["bn1_m"],
                        inputs["bn1_v"])
    sc2, bia2 = bn_fold(inputs["bn2_g"], inputs["bn2_b"], inputs["bn2_m"],
                        inputs["bn2_v"])

    fc1w1 = pack_fc1(inputs["w1_fc1_w"], inputs["w1_fc1_b"])
    fc1w2 = pack_fc1(inputs["w2_fc1_w"], inputs["w2_fc1_b"])

    base = {
        "rwT": f32((np.asarray(inputs["reduce_w"], np.float32).T
                    / (NPIX * SX)).reshape(2, 128, 16)),
        "rb": f32(np.asarray(inputs["reduce_b"]).reshape(16, 1)),
        "fc1wTp1": fc1w1, "fc1wTp2": fc1w2,
        "w2p1": pack_w2(inputs["w1_fc2_w"]),
        "w2p2": pack_w2(inputs["w2_fc2_w"]),
        "bns1": f32((sc1 * SO1 / (SX * SW)).reshape(2, 128, 1)),
        "bnb1": f32((bia1 * SO1).reshape(2, 128, 1)),
        "bns2": f32((sc2 / (SO1 * SW)).reshape(2, 128, 1)),
    }

    # residual with bn2 bias folded in
    xb2 = (x + bia2[None, :, None, None]).astype(BFNP)

    in_maps = []
    for i in range(NCORES):
        m = dict(base)
        sl = slice(i * BL, (i + 1) * BL)
        m["xh8p"] = np.ascontiguousarray(xh[sl])
        m["xl8p"] = np.ascontiguousarray(xl[sl])
        m["xb2"] = np.ascontiguousarray(xb2[sl])
        in_maps.append(m)
    return in_maps


def kernel(**inputs):
    in_maps = prep_inputs(inputs)
    nc = get_program()
    res = bass_utils.run_bass_kernel_spmd(nc, in_maps,
                                          core_ids=list(range(NCORES)))
    out = np.concatenate([r["out4"] for r in res.results], axis=0)
    return out.astype(np.float32)
